# revision 27
# baseline (speedup 1.0000x reference)
"""Trainium2 Bass kernel for nn_L2MLoRAqkv (MoE-routed LoRA QKV projection).

Math (per batch b, expert i = idx[b,0]):
    qkv = x @ W.T + bias
    qkv[:, :D]  += (x @ A_q[i]) @ B_q[i] * SCALE
    qkv[:, -D:] += (x @ A_v[i]) @ B_v[i] * SCALE

Strategy: data-parallel over the batch dim (1 batch per NeuronCore, 8 cores).
On the host we gather each batch's expert and fold the rank-8 LoRA update
into the (transposed) projection weight in float64:
    W_eff[b] = W.T; W_eff[:, :D] += A_q[i] @ B_q[i]; W_eff[:, -D:] += A_v[i] @ B_v[i]
so the device kernel is a single dense GEMM per core:
    Y[4096, 3072] = X[4096, 1024] @ W_eff[1024, 3072] + bias

All GEMM operands move as bf16 (PSUM accumulation stays f32): halves HBM
traffic vs f32 and keeps the PE at 1 cycle/row = 216ns per [128x512]
matmul.  X and W_eff are fully SBUF-resident.  Key structure:
 - Compute is k-outer / n-inner per 128-token tile, so the stationary
   operand (x tile) is identical across 6 consecutive matmuls; per-matmul
   LDWEIGHTS traffic otherwise costs ~43ns/matmul of PE issue rate.
 - The startup-critical data (w[:,n0] + x chunk-0, consumed k-step by
   k-step by the head phase) is host-packed into a [1024,1024] blob whose
   per-k pieces are fully contiguous 256KB DMAs — ~300GB/s instead of the
   ~190GB/s that 1KB-line strided pieces reach.  The blob rides the scalar
   ring while the sync ring streams the rest of w from ring-up; x k-row
   tails (7KB lines) follow on both rings, then the output stores.
   HWDGE FIFO order per ring is the priority mechanism.
 - bias ships as one 12KB row and is replicated across partitions on-chip
   via a ones[1,128].T @ bias_row matmul into PSUM (saves a 1.5MB DMA).
 - ~30 dependency-free warm-up matmuls before the head keep the PE busy
   from engine-up so the HAM clock gate reaches 8/8 (2.4GHz) early.
 - Output tiles coalesce 6 bias-add drains into one [128,3072] bf16 tile
   stored as two half-rows on both rings (3KB lines); the final tile
   stores slice-wise so the tail drains on both rings in parallel.
"""

import os
import sys

import numpy as np

for _p in ("/opt/trn_rl_repo",):
    if _p not in sys.path and os.path.isdir(_p):
        sys.path.insert(0, _p)

B = 8          # batches == cores
T = 4096       # tokens per batch
D = 1024       # model dim (contraction K)
N3 = 3072      # qkv output dim
P = 128        # SBUF partitions
NT = 512       # n-tile (one fp32 PSUM bank)
CH = 512       # tokens covered by the startup blob (chunk 0)
KT = D // P        # 8 k-tiles
NN = N3 // NT      # 6 n-tiles
TT = CH // P       # 4 token sub-tiles in chunk 0
WW = N3 - NT       # w_sb row width (n1..n5)
XW = T - CH        # x_sb row width (chunks 1..7)
BW = 2 * NT        # blob row width (w n0-slice ++ x c0-slice)
SCALE = 8.0 / 8.0

_NC_CACHE = {}


def _build():
    import concourse.tile as tile
    from concourse import bacc, mybir

    bf16 = mybir.dt.bfloat16
    f32 = mybir.dt.float32

    nc = bacc.Bacc(
        "TRN2",
        target_bir_lowering=False,
        debug=False,
        enable_asserts=False,
        num_devices=B,
    )
    xt = nc.dram_tensor("xt", [D, T], bf16, kind="ExternalInput").ap()
    weff = nc.dram_tensor("weff", [D, N3], bf16, kind="ExternalInput").ap()
    blob = nc.dram_tensor("blob", [KT * P, BW], bf16, kind="ExternalInput").ap()
    biasr = nc.dram_tensor("biasr", [1, N3], bf16, kind="ExternalInput").ap()
    y = nc.dram_tensor("y", [T, N3], bf16, kind="ExternalOutput").ap()

    with tile.TileContext(nc) as tc:
        with tc.tile_pool(name="const", bufs=1) as const_pool, \
             tc.tile_pool(name="outp", bufs=6) as out_pool, \
             tc.tile_pool(name="ps", bufs=7, space="PSUM") as psum_pool, \
             tc.tile_pool(name="psb", bufs=1, space="PSUM") as psum_b_pool:

            # head_sb col k*BW..: [w[k, 0:NT] ++ x[k, 0:CH]]
            # w_sb col k*WW..:    w[k, NT:N3]
            # x_sb col k*XW..:    x[k, CH:T]
            head_sb = const_pool.tile([P, KT * BW], bf16)
            w_sb = const_pool.tile([P, KT * WW], bf16)
            x_sb = const_pool.tile([P, KT * XW], bf16)
            bias_sb = const_pool.tile([P, N3], f32)
            bias_row = const_pool.tile([1, N3], bf16)
            ones_sb = const_pool.tile([1, P], bf16)

            # Scalar ring: bias row, then the startup blob piece by piece —
            # each piece is one fully-contiguous 256KB read that unlocks one
            # head k-step.  Sync ring: the rest of w, from ring-up.
            nc.scalar.dma_start(bias_row[:], biasr[:])
            for k in range(KT):
                nc.scalar.dma_start(
                    head_sb[:, k * BW : (k + 1) * BW],
                    blob[k * P : (k + 1) * P, :],
                )
            for k in range(KT):
                nc.sync.dma_start(
                    w_sb[:, k * WW : (k + 1) * WW],
                    weff[k * P : (k + 1) * P, NT:N3],
                )
            # x k-row tails (7KB partition lines) on both rings, then stores.
            for k in range(KT):
                eng = nc.scalar if k % 2 else nc.sync
                eng.dma_start(
                    x_sb[:, k * XW : (k + 1) * XW],
                    xt[k * P : (k + 1) * P, CH:T],
                )

            # One [128, 3072] output tile per token tile: the six bias-add
            # drains fill it slice by slice, then two half-row stores (3KB
            # partition lines) ship it on both rings in parallel.
            obs = {}

            def drain(ps, tg, n):
                if tg not in obs:
                    obs[tg] = out_pool.tile([P, N3], bf16, tag="ob", name="ob")
                ob = obs[tg]
                nc.vector.tensor_add(
                    ob[:, n * NT : (n + 1) * NT],
                    ps[:],
                    bias_sb[:, n * NT : (n + 1) * NT],
                )

            def store(tg, final=False):
                ob = obs.pop(tg)
                if final:
                    # Six slice-stores alternating rings: the first slices
                    # ship while the last drains still run, shrinking the
                    # kernel tail.
                    for n in range(NN):
                        eng = nc.sync if n % 2 else nc.scalar
                        eng.dma_start(
                            y[tg * P : (tg + 1) * P, n * NT : (n + 1) * NT],
                            ob[:, n * NT : (n + 1) * NT],
                        )
                    return
                half = N3 // 2
                nc.scalar.dma_start(
                    y[tg * P : (tg + 1) * P, 0:half], ob[:, 0:half]
                )
                nc.sync.dma_start(
                    y[tg * P : (tg + 1) * P, half:N3], ob[:, half:N3]
                )

            def mm(ps, tg, n, k):
                if tg < TT:
                    xap = head_sb[:, k * BW + NT + tg * P
                                  : k * BW + NT + (tg + 1) * P]
                else:
                    off = k * XW + tg * P - CH
                    xap = x_sb[:, off : off + P]
                if n == 0:
                    wap = head_sb[:, k * BW : k * BW + NT]
                else:
                    wap = w_sb[:, k * WW + (n - 1) * NT : k * WW + n * NT]
                nc.tensor.matmul(
                    ps[:],
                    lhsT=xap,
                    rhs=wap,
                    start=(k == 0),
                    stop=(k == KT - 1),
                )

            # ~30 dependency-free warm-up matmuls on the ones tile keep the
            # PE busy from engine-up (~7us) until the first data lands
            # (~10us), so the HAM clock gate reaches 8/8 early.  Output goes
            # to a dedicated PSUM bank that is never read.
            nc.vector.memset(ones_sb[:], 1.0)
            wub = psum_b_pool.tile([P, P], f32, tag="psb", name="psb")
            for _ in range(30):
                nc.tensor.matmul(
                    wub[:], lhsT=ones_sb[:], rhs=ones_sb[:],
                    start=True, stop=True,
                )

            # Head phase (chunk 0, n=0): k-outer over 4 parallel PSUM groups
            # so the PE consumes blob pieces in exactly DMA arrival order.
            pss = [psum_pool.tile([P, NT], f32, tag="ps", name="ps")
                   for _ in range(TT)]
            for k in range(KT):
                for t in range(TT):
                    mm(pss[t], t, 0, k)

            # Replicate bias across partitions on-chip: ones[1,128].T @
            # bias_row[1,512] fills [128,512].  12KB of HBM instead of 1.5MB,
            # and the PE does it while the head phase wraps up.
            for n in range(NN):
                psb = psum_pool.tile([P, NT], f32, tag="ps", name="ps")
                nc.tensor.matmul(
                    psb[:],
                    lhsT=ones_sb[:],
                    rhs=bias_row[:, n * NT : (n + 1) * NT],
                    start=True,
                    stop=True,
                )
                nc.vector.tensor_copy(bias_sb[:, n * NT : (n + 1) * NT], psb[:])

            for t in range(TT):
                drain(pss[t], t, 0)

            # Everything else: k-outer / n-inner with one PSUM bank per n, so
            # the stationary operand (lhsT = x tile) is identical across the
            # n-consecutive matmuls — the PE's weight path stays quiet.
            def t_block(tg, n_lo):
                pss = [psum_pool.tile([P, NT], f32, tag="ps", name="ps")
                       for _ in range(NN - n_lo)]
                for k in range(KT):
                    for n in range(n_lo, NN):
                        mm(pss[n - n_lo], tg, n, k)
                for n in range(n_lo, NN):
                    drain(pss[n - n_lo], tg, n)
                store(tg, final=(tg == T // P - 1))

            for t in range(TT):
                t_block(t, 1)
            for tg in range(TT, T // P):
                t_block(tg, 0)
    nc.compile()
    return nc


def _get_nc():
    if "v2" not in _NC_CACHE:
        _NC_CACHE["v2"] = _build()
    return _NC_CACHE["v2"]


def _prep_in_maps(inputs):
    import ml_dtypes

    bf16 = ml_dtypes.bfloat16

    x = np.asarray(inputs["x"], dtype=np.float32)
    weight = np.asarray(inputs["weight"], dtype=np.float32)
    bias = np.asarray(inputs["bias"], dtype=np.float32)
    aq = np.asarray(inputs["A_q_pool"], dtype=np.float32)
    bq = np.asarray(inputs["B_q_pool"], dtype=np.float32)
    av = np.asarray(inputs["A_v_pool"], dtype=np.float32)
    bv = np.asarray(inputs["B_v_pool"], dtype=np.float32)
    idx = np.asarray(inputs["idx"]).reshape(B, -1)[:, 0].astype(np.int64)

    wt64 = weight.T.astype(np.float64)  # [D, N3]
    biasr = np.ascontiguousarray(bias.reshape(1, N3))
    xts = x.transpose(0, 2, 1)  # [B, D, T] strided view

    in_maps = []
    for b in range(B):
        i = int(idx[b])
        weff = wt64.copy()
        weff[:, :D] += SCALE * (aq[i].astype(np.float64) @ bq[i].astype(np.float64))
        weff[:, N3 - D:] += SCALE * (av[i].astype(np.float64) @ bv[i].astype(np.float64))
        xtb = np.ascontiguousarray(xts[b]).astype(bf16)
        weffb = weff.astype(np.float32).astype(bf16)
        blobb = np.vstack([
            np.hstack([weffb[k * P : (k + 1) * P, 0:NT],
                       xtb[k * P : (k + 1) * P, 0:CH]])
            for k in range(KT)
        ])
        in_maps.append({
            "xt": xtb,
            "weff": weffb,
            "blob": np.ascontiguousarray(blobb),
            "biasr": biasr.astype(bf16),
        })
    return in_maps


def _run(in_maps, trace=False, **kwargs):
    from concourse.bass_utils import run_bass_kernel_spmd

    nc = _get_nc()
    return run_bass_kernel_spmd(
        nc, in_maps, core_ids=list(range(B)), trace=trace, **kwargs
    )


def kernel(**inputs):
    res = _run(_prep_in_maps(inputs), trace=False)
    return np.stack(
        [np.asarray(r["y"], dtype=np.float32) for r in res.results], axis=0
    )


# revision 28
# speedup vs baseline: 1.0127x; 1.0127x over previous
"""Trainium2 Bass kernel for nn_L2MLoRAqkv (MoE-routed LoRA QKV projection).

Math (per batch b, expert i = idx[b,0]):
    qkv = x @ W.T + bias
    qkv[:, :D]  += (x @ A_q[i]) @ B_q[i] * SCALE
    qkv[:, -D:] += (x @ A_v[i]) @ B_v[i] * SCALE

Strategy: data-parallel over the batch dim (1 batch per NeuronCore, 8 cores).
On the host we gather each batch's expert and fold the rank-8 LoRA update
into the (transposed) projection weight in float64:
    W_eff[b] = W.T; W_eff[:, :D] += A_q[i] @ B_q[i]; W_eff[:, -D:] += A_v[i] @ B_v[i]
so the device kernel is a single dense GEMM per core:
    Y[4096, 3072] = X[4096, 1024] @ W_eff[1024, 3072] + bias

All GEMM operands move as bf16 (PSUM accumulation stays f32): halves HBM
traffic vs f32 and keeps the PE at 1 cycle/row = 216ns per [128x512]
matmul.  X and W_eff are fully SBUF-resident.  Key structure:
 - Compute is k-outer / n-inner per 128-token tile, so the stationary
   operand (x tile) is identical across 6 consecutive matmuls; per-matmul
   LDWEIGHTS traffic otherwise costs ~43ns/matmul of PE issue rate.
 - The startup-critical data (w[:,n0] + x chunk-0, consumed k-step by
   k-step by the head phase) is host-packed into a [1024,1024] blob whose
   per-k pieces are fully contiguous 256KB DMAs — ~300GB/s instead of the
   ~190GB/s that 1KB-line strided pieces reach.  The blob rides the scalar
   ring while the sync ring streams the rest of w from ring-up; x k-row
   tails (7KB lines) follow on both rings, then the output stores.
   HWDGE FIFO order per ring is the priority mechanism.
 - bias ships as one 12KB row and is replicated across partitions on-chip
   via a ones[1,128].T @ bias_row matmul into PSUM (saves a 1.5MB DMA).
 - ~30 dependency-free warm-up matmuls before the head keep the PE busy
   from engine-up so the HAM clock gate reaches 8/8 (2.4GHz) early.
 - Output tiles coalesce 6 bias-add drains into one [128,3072] bf16 tile
   stored as two half-rows on both rings (3KB lines); the final tile
   stores slice-wise so the tail drains on both rings in parallel.
"""

import os
import sys

import numpy as np

for _p in ("/opt/trn_rl_repo",):
    if _p not in sys.path and os.path.isdir(_p):
        sys.path.insert(0, _p)

B = 8          # batches == cores
T = 4096       # tokens per batch
D = 1024       # model dim (contraction K)
N3 = 3072      # qkv output dim
P = 128        # SBUF partitions
NT = 512       # n-tile (one fp32 PSUM bank)
CH = 512       # tokens covered by the startup blob (chunk 0)
KT = D // P        # 8 k-tiles
NN = N3 // NT      # 6 n-tiles
TT = CH // P       # 4 token sub-tiles in chunk 0
WW = N3 - NT       # w_sb row width (n1..n5)
XW = T - CH        # x_sb row width (chunks 1..7)
BW = 2 * NT        # blob row width (w n0-slice ++ x c0-slice)
SCALE = 8.0 / 8.0

_NC_CACHE = {}


def _build():
    import concourse.tile as tile
    from concourse import bacc, mybir

    bf16 = mybir.dt.bfloat16
    f32 = mybir.dt.float32

    nc = bacc.Bacc(
        "TRN2",
        target_bir_lowering=False,
        debug=False,
        enable_asserts=False,
        num_devices=B,
    )
    xt = nc.dram_tensor("xt", [D, T], bf16, kind="ExternalInput").ap()
    weff = nc.dram_tensor("weff", [D, N3], bf16, kind="ExternalInput").ap()
    blob = nc.dram_tensor("blob", [KT * P, BW], bf16, kind="ExternalInput").ap()
    biasr = nc.dram_tensor("biasr", [1, N3], bf16, kind="ExternalInput").ap()
    y = nc.dram_tensor("y", [T, N3], bf16, kind="ExternalOutput").ap()

    with tile.TileContext(nc) as tc:
        with tc.tile_pool(name="const", bufs=1) as const_pool, \
             tc.tile_pool(name="outp", bufs=6) as out_pool, \
             tc.tile_pool(name="ps", bufs=7, space="PSUM") as psum_pool, \
             tc.tile_pool(name="psb", bufs=1, space="PSUM") as psum_b_pool:

            # head_sb col k*BW..: [w[k, 0:NT] ++ x[k, 0:CH]]
            # w_sb col k*WW..:    w[k, NT:N3]
            # x_sb col k*XW..:    x[k, CH:T]
            head_sb = const_pool.tile([P, KT * BW], bf16)
            w_sb = const_pool.tile([P, KT * WW], bf16)
            x_sb = const_pool.tile([P, KT * XW], bf16)
            bias_sb = const_pool.tile([P, N3], f32)
            bias_row = const_pool.tile([1, N3], bf16)
            ones_sb = const_pool.tile([1, P], bf16)

            # Scalar ring: bias row, then the startup blob piece by piece —
            # each piece is one fully-contiguous 256KB read that unlocks one
            # head k-step.  Sync ring: the rest of w, from ring-up.
            nc.scalar.dma_start(bias_row[:], biasr[:])
            for k in range(KT):
                eng = nc.scalar if k % 2 else nc.sync
                eng.dma_start(
                    head_sb[:, k * BW : (k + 1) * BW],
                    blob[k * P : (k + 1) * P, :],
                )
            for k in range(KT):
                eng = nc.sync if k % 2 else nc.scalar
                eng.dma_start(
                    w_sb[:, k * WW : (k + 1) * WW],
                    weff[k * P : (k + 1) * P, NT:N3],
                )
            # x k-row tails (7KB partition lines) on both rings, then stores.
            for k in range(KT):
                eng = nc.scalar if k % 2 else nc.sync
                eng.dma_start(
                    x_sb[:, k * XW : (k + 1) * XW],
                    xt[k * P : (k + 1) * P, CH:T],
                )

            # One [128, 3072] output tile per token tile: the six bias-add
            # drains fill it slice by slice, then two half-row stores (3KB
            # partition lines) ship it on both rings in parallel.
            obs = {}

            def drain(ps, tg, n):
                if tg not in obs:
                    obs[tg] = out_pool.tile([P, N3], bf16, tag="ob", name="ob")
                ob = obs[tg]
                nc.vector.tensor_add(
                    ob[:, n * NT : (n + 1) * NT],
                    ps[:],
                    bias_sb[:, n * NT : (n + 1) * NT],
                )

            def store(tg, final=False):
                ob = obs.pop(tg)
                if final:
                    # Six slice-stores alternating rings: the first slices
                    # ship while the last drains still run, shrinking the
                    # kernel tail.
                    for n in range(NN):
                        eng = nc.sync if n % 2 else nc.scalar
                        eng.dma_start(
                            y[tg * P : (tg + 1) * P, n * NT : (n + 1) * NT],
                            ob[:, n * NT : (n + 1) * NT],
                        )
                    return
                half = N3 // 2
                nc.scalar.dma_start(
                    y[tg * P : (tg + 1) * P, 0:half], ob[:, 0:half]
                )
                nc.sync.dma_start(
                    y[tg * P : (tg + 1) * P, half:N3], ob[:, half:N3]
                )

            def mm(ps, tg, n, k):
                if tg < TT:
                    xap = head_sb[:, k * BW + NT + tg * P
                                  : k * BW + NT + (tg + 1) * P]
                else:
                    off = k * XW + tg * P - CH
                    xap = x_sb[:, off : off + P]
                if n == 0:
                    wap = head_sb[:, k * BW : k * BW + NT]
                else:
                    wap = w_sb[:, k * WW + (n - 1) * NT : k * WW + n * NT]
                nc.tensor.matmul(
                    ps[:],
                    lhsT=xap,
                    rhs=wap,
                    start=(k == 0),
                    stop=(k == KT - 1),
                )

            # ~30 dependency-free warm-up matmuls on the ones tile keep the
            # PE busy from engine-up (~7us) until the first data lands
            # (~10us), so the HAM clock gate reaches 8/8 early.  Output goes
            # to a dedicated PSUM bank that is never read.
            nc.vector.memset(ones_sb[:], 1.0)
            wub = psum_b_pool.tile([P, P], f32, tag="psb", name="psb")
            for _ in range(30):
                nc.tensor.matmul(
                    wub[:], lhsT=ones_sb[:], rhs=ones_sb[:],
                    start=True, stop=True,
                )

            # Head phase (chunk 0, n=0): k-outer over 4 parallel PSUM groups
            # so the PE consumes blob pieces in exactly DMA arrival order.
            pss = [psum_pool.tile([P, NT], f32, tag="ps", name="ps")
                   for _ in range(TT)]
            for k in range(KT):
                for t in range(TT):
                    mm(pss[t], t, 0, k)

            # Replicate bias across partitions on-chip: ones[1,128].T @
            # bias_row[1,512] fills [128,512].  12KB of HBM instead of 1.5MB,
            # and the PE does it while the head phase wraps up.
            for n in range(NN):
                psb = psum_pool.tile([P, NT], f32, tag="ps", name="ps")
                nc.tensor.matmul(
                    psb[:],
                    lhsT=ones_sb[:],
                    rhs=bias_row[:, n * NT : (n + 1) * NT],
                    start=True,
                    stop=True,
                )
                nc.vector.tensor_copy(bias_sb[:, n * NT : (n + 1) * NT], psb[:])

            for t in range(TT):
                drain(pss[t], t, 0)

            # Everything else: k-outer / n-inner with one PSUM bank per n, so
            # the stationary operand (lhsT = x tile) is identical across the
            # n-consecutive matmuls — the PE's weight path stays quiet.
            def t_block(tg, n_lo):
                pss = [psum_pool.tile([P, NT], f32, tag="ps", name="ps")
                       for _ in range(NN - n_lo)]
                for k in range(KT):
                    for n in range(n_lo, NN):
                        mm(pss[n - n_lo], tg, n, k)
                for n in range(n_lo, NN):
                    drain(pss[n - n_lo], tg, n)
                store(tg, final=(tg == T // P - 1))

            for t in range(TT):
                t_block(t, 1)
            for tg in range(TT, T // P):
                t_block(tg, 0)
    nc.compile()
    return nc


def _get_nc():
    if "v2" not in _NC_CACHE:
        _NC_CACHE["v2"] = _build()
    return _NC_CACHE["v2"]


def _prep_in_maps(inputs):
    import ml_dtypes

    bf16 = ml_dtypes.bfloat16

    x = np.asarray(inputs["x"], dtype=np.float32)
    weight = np.asarray(inputs["weight"], dtype=np.float32)
    bias = np.asarray(inputs["bias"], dtype=np.float32)
    aq = np.asarray(inputs["A_q_pool"], dtype=np.float32)
    bq = np.asarray(inputs["B_q_pool"], dtype=np.float32)
    av = np.asarray(inputs["A_v_pool"], dtype=np.float32)
    bv = np.asarray(inputs["B_v_pool"], dtype=np.float32)
    idx = np.asarray(inputs["idx"]).reshape(B, -1)[:, 0].astype(np.int64)

    wt64 = weight.T.astype(np.float64)  # [D, N3]
    biasr = np.ascontiguousarray(bias.reshape(1, N3))
    xts = x.transpose(0, 2, 1)  # [B, D, T] strided view

    in_maps = []
    for b in range(B):
        i = int(idx[b])
        weff = wt64.copy()
        weff[:, :D] += SCALE * (aq[i].astype(np.float64) @ bq[i].astype(np.float64))
        weff[:, N3 - D:] += SCALE * (av[i].astype(np.float64) @ bv[i].astype(np.float64))
        xtb = np.ascontiguousarray(xts[b]).astype(bf16)
        weffb = weff.astype(np.float32).astype(bf16)
        blobb = np.vstack([
            np.hstack([weffb[k * P : (k + 1) * P, 0:NT],
                       xtb[k * P : (k + 1) * P, 0:CH]])
            for k in range(KT)
        ])
        in_maps.append({
            "xt": xtb,
            "weff": weffb,
            "blob": np.ascontiguousarray(blobb),
            "biasr": biasr.astype(bf16),
        })
    return in_maps


def _run(in_maps, trace=False, **kwargs):
    from concourse.bass_utils import run_bass_kernel_spmd

    nc = _get_nc()
    return run_bass_kernel_spmd(
        nc, in_maps, core_ids=list(range(B)), trace=trace, **kwargs
    )


def kernel(**inputs):
    res = _run(_prep_in_maps(inputs), trace=False)
    return np.stack(
        [np.asarray(r["y"], dtype=np.float32) for r in res.results], axis=0
    )
